# revision 27
# baseline (speedup 1.0000x reference)
"""Causal self-attention with RoPE on 8 Trainium2 NeuronCores.

Sharding: batch (2) x head-groups (4 of 4 heads) -> 8 cores. Each core
projects its batch-half of x against its 4 heads' slice of w_qkv, runs
causal flash-style attention for those heads, and applies its slice of
w_out, producing a partial [2048, 1024] output. Host sums the 4 partials
per batch.

All matmul operands are bf16 (fp32 HIGH mode streams ~2 cycles/col on PE;
bf16 streams 1 col/cycle and enables FWL weight loads). PSUM accumulation
stays fp32. Measured-precision budget: harness gate is 2e-2 max-normalized.

On-device layouts (per core, S=2048, 4 heads, hd=64):
  xT      [1024, 2048] bf16  x[g].T               (d on partitions, chunked)
  QT/KT   2 x 4 x [128, 512] q^T/k^T per t-chunk, rows = 2 heads x 64, RoPE'd
  Vsb     4 x [128, 4*260]  v in [k, d] layout, key-blocks of
                            [v_h0|1|v_h1|1|v_h2|1|v_h3|1] (ones col -> den).
                            Produced DIRECTLY in [t, d] layout by swapping
                            matmul operands (lhsT = x chunk, rhs = w_v^T):
                            no PE transposes needed.
  scores  S^T [k, q] via matmul(lhsT=KT slice, rhs=QT slice); both heads of a
          pair write one shared [128,1024] psum tile (A cols 0:512, B cols
          512:1024) so a single ACT exp covers both; causal handled block-wise
          with ONE merged 3D-AP mask multiply for both streams
  ctx^T   accumulated [65, 512] psum per (head, q-chunk); row 64 = denominator
          (ones column appended to V); reciprocal read straight from PSUM,
          GpSimd partition-broadcast, one DVE multiply into bf16 ctxc
  out     ctxT [256, 2048] x w_outT -> partial [2048, 1024] bf16

Projection, attention, and output projection are emitted as interleaved unit
streams (attention(qc) with projection(qc+1), out-proj(qc) with attention
(qc+1)) so PE stays dense. Within an attention unit the AVs of the previous
block precede the scores (AV-B first: its exp wait subsumes AV-A's), leaving
the two K=64 score matmuls wait-free and adjacent — they execute concurrently
on PE row-groups 0/64. A burst of dependency-free warmup matmuls at kernel
start keeps the PE HAM busy during the initial DMA so real work runs at
K=8/8 (2.4 GHz) from the first tile.
"""

import numpy as np

DIM = 1024
NUM_HEADS = 16
HEAD_DIM = 64
ROPE_BASE = 10000.0
B = 2
S = 2048
N_CORES = 8
HPC = 4            # heads per core
GROUPS = 2         # batch groups
CPG = N_CORES // GROUPS  # cores per group
TC = 512           # t-chunk (tokens per projection chunk)
NTC = S // TC      # 4
KB = 128           # key block
NKB = S // KB      # 16
VW = 65            # v block width per head (64 + ones col)
VBW = HPC * VW     # 260
N_WARMUP = 44      # dependency-free PE warmup matmuls

_compiled = None


def _build_nc(debug_dump=False):
    import concourse.tile as tile
    from concourse import bacc, mybir
    from contextlib import ExitStack

    f32 = mybir.dt.float32
    bf16 = mybir.dt.bfloat16
    AF = mybir.ActivationFunctionType

    nc = bacc.Bacc("TRN2", target_bir_lowering=False, debug=False,
                   num_devices=N_CORES)

    xT = nc.dram_tensor("xT", [DIM, S], bf16, kind="ExternalInput").ap()
    wqkT = nc.dram_tensor("wqkT", [128, 8 * 512], bf16, kind="ExternalInput").ap()
    wvT = nc.dram_tensor("wvT", [128, 8 * 256], bf16, kind="ExternalInput").ap()
    woT = nc.dram_tensor("woT", [128, 2 * 1024], bf16, kind="ExternalInput").ap()
    cosT = nc.dram_tensor("cosT", [128, S], bf16, kind="ExternalInput").ap()
    sinT = nc.dram_tensor("sinT", [128, S], bf16, kind="ExternalInput").ap()
    perm = nc.dram_tensor("perm", [128, 128], bf16, kind="ExternalInput").ap()
    tri2 = nc.dram_tensor("tri2", [128, 256], bf16, kind="ExternalInput").ap()
    out_p = nc.dram_tensor("out_p", [S, DIM], bf16, kind="ExternalOutput").ap()
    if debug_dump:
        dbg_qt0 = nc.dram_tensor("dbg_qt0", [128, S], bf16, kind="ExternalOutput").ap()
        dbg_kt0 = nc.dram_tensor("dbg_kt0", [128, S], bf16, kind="ExternalOutput").ap()
        dbg_vsb = nc.dram_tensor("dbg_vsb", [128, NKB * VBW], bf16, kind="ExternalOutput").ap()
        dbg_ctx0 = nc.dram_tensor("dbg_ctx0", [128, S], bf16, kind="ExternalOutput").ap()
        dbg_cps = nc.dram_tensor("dbg_cps", [128, S], f32, kind="ExternalOutput").ap()
        dbg_bcr = nc.dram_tensor("dbg_bcr", [128, S], f32, kind="ExternalOutput").ap()

    with tile.TileContext(nc) as tc:
        with ExitStack() as ctx:
            big = ctx.enter_context(tc.tile_pool(name="big", bufs=1))

            # warmup tile first: memset has no DMA deps, so the warmup
            # matmuls below run while the first DMAs are still in flight
            warm_sb = big.tile([128, 128], bf16, tag="warm")
            nc.gpsimd.memset(warm_sb[:], 0.0)

            # DMA order matters: the queue drains serially, so interleave the
            # per-c8 wqk chunks with the first x chunks (the startup critical
            # path), then rope tables, then later-needed weights.
            # DMA order matters: two hardware DMA queues exist (sync + scalar).
            # Weights go on the scalar queue (idle at startup), x chunks on
            # sync, so the startup critical path drains two streams in
            # parallel.
            wqk_sb = [big.tile([128, 512], bf16, tag=f"wqk{c8}", name=f"wqk{c8}")
                      for c8 in range(8)]
            x0ts = []
            for c8 in range(8):
                nc.sync.dma_start(wqk_sb[c8][:], wqkT[:, c8 * 512:(c8 + 1) * 512])
                xt_t = big.tile([128, TC], bf16, tag=f"x0_{c8}", name=f"x{c8}_0")
                nc.sync.dma_start(xt_t[:], xT[c8 * 128:(c8 + 1) * 128, 0:TC])
                x0ts.append(xt_t)
            perm_sb = big.tile([128, 128], bf16, tag="perm")
            nc.sync.dma_start(perm_sb[:], perm[:])
            cos_sb = big.tile([128, S], bf16, tag="cos")
            nc.sync.dma_start(cos_sb[:], cosT[:])
            sin_sb = big.tile([128, S], bf16, tag="sin")
            nc.sync.dma_start(sin_sb[:], sinT[:])
            wv_sb = big.tile([128, 8 * 256], bf16, tag="wv")
            nc.sync.dma_start(wv_sb[:], wvT[:])
            tri2_sb = big.tile([128, 256], bf16, tag="tri2")
            nc.sync.dma_start(tri2_sb[:], tri2[:])
            wo_sb = big.tile([128, 2 * 1024], bf16, tag="wo")
            nc.sync.dma_start(wo_sb[:], woT[:])

            # per-chunk persistent tiles so phase 2 can start while phase 1
            # is still projecting later chunks
            qt = [[big.tile([128, TC], bf16, tag=f"qt{i}_{j}", name=f"qt{i}_{j}")
                   for j in range(NTC)] for i in range(2)]
            kt = [[big.tile([128, TC], bf16, tag=f"kt{i}_{j}", name=f"kt{i}_{j}")
                   for j in range(NTC)] for i in range(2)]
            vsb = [big.tile([128, 4 * VBW], bf16, tag=f"vsb{j}", name=f"vsb{j}")
                   for j in range(NTC)]
            ctxc = [[big.tile([128, TC], bf16, tag=f"ctx{i}_{j}", name=f"ctxc{i}_{j}")
                     for j in range(NTC)] for i in range(2)]

            if debug_dump:
                dbg_cps_sb = big.tile([128, S], f32, tag="dbgcps")
                dbg_bcr_sb = big.tile([128, S], f32, tag="dbgbcr")

            # ones columns interleaved in v blocks (denominator trick)
            ones128 = big.tile([128, 16], bf16, tag="ones128")
            nc.gpsimd.memset(ones128[:], 1.0)
            for j in range(NTC):
                nc.vector.tensor_copy(vsb[j][:, 64::VW], ones128[:])

            # ---- Fused phase 1 (projection) + phase 2 (attention) ----
            # attention for q-chunk qc only needs qt/kt/vsb chunks <= qc, so
            # attention(qc) is emitted interleaved with projection(qc+1).
            scale = float(HEAD_DIM) ** -0.5
            with (
                tc.tile_pool(name="p1x", bufs=2) as p1x,
                tc.tile_pool(name="p1sb", bufs=3) as p1sb,
                tc.tile_pool(name="p2sb", bufs=6) as p2sb,
                tc.tile_pool(name="pps", bufs=2, space="PSUM") as pps,
                tc.tile_pool(name="sps", bufs=2, space="PSUM") as sps,
                tc.tile_pool(name="ctxps", bufs=2, space="PSUM") as ctxps,
            ):
                rtps = pps

                # PE warmup: no data deps -> runs immediately, keeps the HAM
                # activity window busy while the first DMAs land
                warm_ps = pps.tile([128, 512], f32, tag="p", name="warm_ps")
                for _ in range(N_WARMUP):
                    nc.tensor.matmul(warm_ps[:, 0:128], warm_sb[:], warm_sb[:],
                                     start=True, stop=True)

                def p1_dma(tci):
                    if tci == 0:
                        return x0ts
                    xts = []
                    for c8 in range(8):
                        xt_t = p1x.tile([128, TC], bf16, tag=f"x{c8}", name=f"x{c8}_{tci}")
                        nc.sync.dma_start(
                            xt_t[:], xT[c8 * 128:(c8 + 1) * 128, tci * TC:(tci + 1) * TC])
                        xts.append(xt_t)
                    return xts

                def p1_qk_unit(xts, tci, blk):
                    cosw = cos_sb[:, tci * TC:(tci + 1) * TC]
                    sinw = sin_sb[:, tci * TC:(tci + 1) * TC]
                    qk_ps = pps.tile([128, TC], f32, tag="p", name=f"qk{tci}_{blk}")
                    for c8 in range(8):
                        nc.tensor.matmul(
                            qk_ps[:],
                            wqk_sb[c8][:, blk * 128:blk * 128 + 128],
                            xts[c8][:],
                            start=(c8 == 0), stop=(c8 == 7))
                    raw = p1sb.tile([128, TC], bf16, tag="raw", name=f"raw{tci}_{blk}")
                    # ACT drains psum (fast psum reads) so the projection's
                    # psum recycling doesn't ride on DVE's queue latency
                    nc.scalar.copy(raw[:], qk_ps[:])
                    rot_ps = rtps.tile([128, TC], f32, tag="p", name=f"rot{tci}_{blk}")
                    nc.tensor.matmul(rot_ps[:], perm_sb[:], raw[:], start=True, stop=True)
                    t1 = p1sb.tile([128, TC], bf16, tag="t1", name=f"t1_{tci}_{blk}")
                    nc.vector.tensor_mul(t1[:], raw[:], cosw)
                    t2 = p1sb.tile([128, TC], bf16, tag="t2", name=f"t2_{tci}_{blk}")
                    nc.vector.tensor_mul(t2[:], rot_ps[:], sinw)
                    dest = (qt if blk < 2 else kt)[blk % 2][tci]
                    nc.vector.tensor_add(dest[:], t1[:], t2[:])

                def p1_v_unit(xts, tci, tsub):
                    # V^T directly: out[t, vd] = (x chunk)^T @ w_v^T chunk,
                    # accumulating K=1024 over the 8 din chunks. lhsT is the
                    # x data (stationary), rhs the weight slice (moving).
                    v_ps = pps.tile([128, 256], f32, tag="p", name=f"vp{tci}_{tsub}")
                    for c8 in range(8):
                        nc.tensor.matmul(
                            v_ps[:],
                            xts[c8][:, tsub * 128:(tsub + 1) * 128],
                            wv_sb[:, c8 * 256:(c8 + 1) * 256],
                            start=(c8 == 0), stop=(c8 == 7))
                    # one strided cast into the per-head interleaved v layout
                    dst = vsb[tci][:, tsub * VBW:(tsub + 1) * VBW]
                    dst3 = dst.rearrange("p (h d) -> p h d", d=VW)[:, :, 0:64]
                    src3 = v_ps[:].rearrange("p (h d) -> p h d", d=64)
                    nc.vector.tensor_copy(dst3, src3)

                def p1_units(tci):
                    xts = p1_dma(tci)
                    units = []
                    for blk in range(4):
                        units.append(lambda b=blk: p1_qk_unit(xts, tci, b))
                    for tsub in range(4):
                        units.append(lambda t=tsub: p1_v_unit(xts, tci, t))
                    return units

                def att_scores(streams, kb):
                    """Both streams' score matmuls for k-block kb into ONE
                    shared psum tile (A cols 0:n, B cols 512:512+n). The two
                    K=64 matmuls at partition bases 0/64 are adjacent and
                    wait-free -> they run concurrently on PE row-groups,
                    restoring full SBUF-stream rate. One exp covers both."""
                    st0 = streams[0]
                    qc, d0 = st0["qc"], st0["qc"] * 4
                    n0 = max(0, 128 * (kb - d0))
                    n = 512 - n0
                    s_ps = sps.tile([128, 1024], f32, tag="s",
                                    name=f"s{st0['h']}_{qc}_{kb}")
                    e_sb = p2sb.tile([128, 1024], bf16, tag="e",
                                     name=f"e{st0['h']}_{qc}_{kb}", bufs=4)
                    for si_, st in enumerate(streams):
                        ti, po = st["ti"], st["po"]
                        off = 512 * si_
                        nc.tensor.matmul(
                            s_ps[:, off:off + n],
                            kt[ti][kb // 4][po:po + 64, (kb % 4) * 128:(kb % 4) * 128 + 128],
                            qt[ti][qc][po:po + 64, n0:512],
                            start=True, stop=True)
                    if n == 512:
                        nc.scalar.activation(e_sb[:, 0:1024], s_ps[:, 0:1024],
                                             AF.Exp, scale=scale)
                    else:
                        # one ACT over both streams' diag regions via 3D APs
                        ev = e_sb[:].rearrange("p (s q) -> p s q", s=2)[:, :, 0:n]
                        sv = s_ps[:].rearrange("p (s q) -> p s q", s=2)[:, :, 0:n]
                        nc.scalar.activation(ev, sv, AF.Exp, scale=scale)
                    if kb >= d0:
                        # one merged mask multiply for both streams: view the
                        # two 128-col diag regions as a [128, 2, 128] AP
                        ev = e_sb[:].rearrange("p (s q) -> p s q", s=2)[:, :, 0:128]
                        tv = tri2_sb[:].rearrange("p (s q) -> p s q", s=2)
                        nc.vector.tensor_mul(ev, ev, tv)
                    for si_, st in enumerate(streams):
                        st["pend"] = (kb, 512 * si_, n0, n, e_sb)

                def att_av(st):
                    h, qc = st["h"], st["qc"]
                    nkb = qc * 4 + 4
                    kb, o, n0, n, e_sb = st["pend"]
                    nc.tensor.matmul(
                        st["ctx"][0:65, n0:512],
                        vsb[kb // 4][:, (kb % 4) * VBW + VW * h:(kb % 4) * VBW + VW * h + VW],
                        e_sb[:, o:o + n],
                        start=(kb == 0), stop=(kb == nkb - 1))

                def att_finish(st):
                    h, qc, ti, po = st["h"], st["qc"], st["ti"], st["po"]
                    den = p2sb.tile([1, 512], f32, tag="den", name=f"den{h}_{qc}", bufs=2)
                    nc.vector.tensor_copy(den[:], st["ctx"][64:65, :])
                    rden = p2sb.tile([1, 512], f32, tag="rden", name=f"rden{h}_{qc}", bufs=2)
                    nc.vector.reciprocal_approx_fast(rden[:], den[:])
                    bc_r = p2sb.tile([64, 512], f32, tag="bcr", name=f"bcr{h}_{qc}", bufs=2)
                    nc.gpsimd.partition_broadcast(bc_r[:], rden[:])
                    if debug_dump and h == 0:
                        nc.vector.tensor_copy(
                            dbg_cps_sb[0:65, qc * 512:(qc + 1) * 512], st["ctx"][0:65, :])
                        nc.vector.tensor_copy(
                            dbg_bcr_sb[0:64, qc * 512:(qc + 1) * 512], bc_r[:])
                    nc.vector.tensor_mul(
                        ctxc[ti][qc][po:po + 64, :], st["ctx"][0:64, :], bc_r[:])

                def att_units(qc):
                    """Units for all 4 heads at q-chunk qc, two streams each.
                    AVs of pair pb-1 (stream B first: its exp wait subsumes
                    stream A's) precede scores of pb, so score matmuls carry
                    no fresh waits -> row-group pairing at bases 0/64."""
                    units = []
                    for hp in range(2):
                        streams = [{
                            "h": h, "qc": qc, "ti": h // 2, "po": 64 * (h % 2),
                        } for h in (2 * hp, 2 * hp + 1)]

                        def mk_start(strs=streams, q=qc):
                            def u():
                                # alloc B first: B's AVs run first and B's
                                # finish is emitted first, so the pool buf
                                # that frees earliest is the one the next
                                # pair needs earliest
                                for st in reversed(strs):
                                    st["ctx"] = ctxps.tile(
                                        [65, 512], f32, tag="ctx",
                                        name=f"cps{st['h']}_{q}")
                                    st["pend"] = None
                            return u
                        units.append(mk_start())

                        nkb = qc * 4 + 4
                        for kb in range(nkb):
                            def mk_unit(strs=streams, k=kb):
                                def u():
                                    if strs[0]["pend"] is not None:
                                        att_av(strs[1])  # B first: its wait
                                        att_av(strs[0])  # subsumes A's
                                    att_scores(strs, k)
                                return u
                            units.append(mk_unit())

                        def mk_tail(strs=streams):
                            def u():
                                att_av(strs[1])
                                att_av(strs[0])
                                att_finish(strs[1])
                                att_finish(strs[0])
                            return u
                        units.append(mk_tail())
                    return units

                osb_cur = {}

                def out_unit(tt, ec):
                    qc = tt // 4
                    o_ps = pps.tile([128, 512], f32, tag="p", name=f"o{tt}_{ec}")
                    for dc in range(2):
                        nc.tensor.matmul(
                            o_ps[:],
                            ctxc[dc][qc][:, (tt % 4) * 128:(tt % 4) * 128 + 128],
                            wo_sb[:, dc * 1024 + ec * 512:dc * 1024 + ec * 512 + 512],
                            start=(dc == 0), stop=(dc == 1))
                    if ec == 0:
                        osb_cur[tt] = p2sb.tile([128, 1024], bf16, tag="osb",
                                                name=f"ob{tt}", bufs=2)
                    o_sb = osb_cur[tt]
                    if qc == 3 and ec == 0:
                        # tail: exp work is over, ACT is idle -> split the
                        # serial cast chain across both engines
                        nc.scalar.copy(o_sb[:, 0:512], o_ps[:])
                    else:
                        nc.vector.tensor_copy(o_sb[:, ec * 512:(ec + 1) * 512], o_ps[:])
                    if qc == 3:
                        # per-half DMAs so the last transfer is small
                        nc.sync.dma_start(
                            out_p[tt * 128:(tt + 1) * 128, ec * 512:(ec + 1) * 512],
                            o_sb[:, ec * 512:(ec + 1) * 512])
                    elif ec == 1:
                        # one 256KB DMA per 128-row block (contiguous in dram)
                        nc.sync.dma_start(out_p[tt * 128:(tt + 1) * 128, :], o_sb[:])

                def out_units(qc):
                    return [lambda t=tt, e=ec: out_unit(t, e)
                            for tt in range(4 * qc, 4 * qc + 4) for ec in range(2)]

                def run_interleaved(a_units, b_units):
                    na, nb = len(a_units), len(b_units)
                    ia = ib = 0
                    while ia < na or ib < nb:
                        if ib >= nb or (ia < na and ia * nb <= ib * na):
                            a_units[ia](); ia += 1
                        else:
                            b_units[ib](); ib += 1

                run_interleaved(p1_units(0), [])
                run_interleaved(p1_units(1), att_units(0))
                run_interleaved(p1_units(2), att_units(1) + out_units(0))
                run_interleaved(p1_units(3), att_units(2) + out_units(1))
                run_interleaved(att_units(3), out_units(2))
                for u in out_units(3):
                    u()

            if debug_dump:
                for j in range(NTC):
                    nc.sync.dma_start(dbg_qt0[:, j * TC:(j + 1) * TC], qt[0][j][:])
                    nc.sync.dma_start(dbg_kt0[:, j * TC:(j + 1) * TC], kt[0][j][:])
                    nc.sync.dma_start(dbg_vsb[:, j * 4 * VBW:(j + 1) * 4 * VBW],
                                      vsb[j][:])
                    nc.sync.dma_start(dbg_ctx0[:, j * TC:(j + 1) * TC],
                                      ctxc[0][j][:])
                nc.sync.dma_start(dbg_cps[:], dbg_cps_sb[:])
                nc.sync.dma_start(dbg_bcr[:], dbg_bcr_sb[:])

    nc.compile()
    return nc


def _rope_tables():
    inv_freq = 1.0 / (ROPE_BASE ** (np.arange(0, HEAD_DIM, 2, dtype=np.float32) / HEAD_DIM))
    t = np.arange(S, dtype=np.float32)
    freqs = np.outer(t, inv_freq)                      # (S, 32)
    emb = np.concatenate([freqs, freqs], axis=-1)      # (S, 64)
    cos = np.cos(emb).astype(np.float32).T             # (64, S)
    sin = np.sin(emb).astype(np.float32).T
    return np.tile(cos, (2, 1)), np.tile(sin, (2, 1))  # (128, S)


def _perm_mat():
    p = np.zeros((128, 128), dtype=np.float32)
    for base in (0, 64):
        for d in range(32):
            p[base + d + 32, base + d] = -1.0          # rot[d] = -q[d+32]
        for d in range(32, 64):
            p[base + d - 32, base + d] = 1.0           # rot[d] = q[d-32]
    return p


def _bf16():
    from concourse import mybir
    return mybir.dt.np(mybir.dt.bfloat16)


def core_inputs(c, x, w_qkv, w_out, cos2, sin2, perm_np, tri2_np):
    bf = _bf16()
    g = c // CPG
    hs = [HPC * (c % CPG) + i for i in range(HPC)]
    xTg = np.ascontiguousarray(x[g].T).astype(bf)                        # (1024, 2048)

    qrows = np.concatenate([w_qkv[h * 64:(h + 1) * 64] for h in hs])     # (256, 1024)
    krows = np.concatenate([w_qkv[DIM + h * 64:DIM + (h + 1) * 64] for h in hs])
    vrows = np.concatenate([w_qkv[2 * DIM + h * 64:2 * DIM + (h + 1) * 64] for h in hs])
    wqk = np.concatenate([qrows, krows])                                 # (512, 1024)
    wqkT = np.ascontiguousarray(
        wqk.reshape(512, 8, 128).transpose(2, 1, 0).reshape(128, 8 * 512)).astype(bf)
    wvT = np.ascontiguousarray(
        vrows.reshape(256, 8, 128).transpose(2, 1, 0).reshape(128, 8 * 256)).astype(bf)

    didx = np.concatenate([np.arange(h * 64, (h + 1) * 64) for h in hs])  # (256,)
    woTh = w_out[:, didx].T                                               # (256, 1024)
    woT = np.ascontiguousarray(
        woTh.reshape(2, 128, DIM).transpose(1, 0, 2).reshape(128, 2 * DIM)).astype(bf)

    return {
        "xT": xTg, "wqkT": wqkT, "wvT": wvT, "woT": woT,
        "cosT": cos2, "sinT": sin2, "perm": perm_np, "tri2": tri2_np,
    }


def make_in_maps(x, w_qkv, w_out):
    bf = _bf16()
    x = np.asarray(x, dtype=np.float32)
    w_qkv = np.asarray(w_qkv, dtype=np.float32)
    w_out = np.asarray(w_out, dtype=np.float32)
    cos2, sin2 = _rope_tables()
    cos2 = cos2.astype(bf)
    sin2 = sin2.astype(bf)
    perm_np = _perm_mat().astype(bf)
    k_idx = np.arange(128)[:, None]
    q_idx = np.arange(128)[None, :]
    tri_np = (q_idx >= k_idx).astype(np.float32)
    tri2_np = np.concatenate([tri_np, tri_np], axis=1).astype(bf)        # (128, 256)
    return [core_inputs(c, x, w_qkv, w_out, cos2, sin2, perm_np, tri2_np)
            for c in range(N_CORES)]


def get_compiled():
    global _compiled
    if _compiled is None:
        _compiled = _build_nc()
    return _compiled


def gather(results):
    out = np.empty((B, S, DIM), dtype=np.float32)
    for g in range(GROUPS):
        acc = results[g * CPG]["out_p"].astype(np.float32)
        for c in range(g * CPG + 1, (g + 1) * CPG):
            acc += results[c]["out_p"].astype(np.float32)
        out[g] = acc
    return out


def kernel(x, w_qkv, w_out):
    from concourse.bass_utils import run_bass_kernel_spmd
    nc = get_compiled()
    in_maps = make_in_maps(x, w_qkv, w_out)
    res = run_bass_kernel_spmd(nc, in_maps, list(range(N_CORES)))
    return gather(res.results)


# revision 28
# speedup vs baseline: 1.0194x; 1.0194x over previous
"""Causal self-attention with RoPE on 8 Trainium2 NeuronCores.

Sharding: batch (2) x head-groups (4 of 4 heads) -> 8 cores. Each core
projects its batch-half of x against its 4 heads' slice of w_qkv, runs
causal flash-style attention for those heads, and applies its slice of
w_out, producing a partial [2048, 1024] output. Host sums the 4 partials
per batch.

All matmul operands are bf16 (fp32 HIGH mode streams ~2 cycles/col on PE;
bf16 streams 1 col/cycle and enables FWL weight loads). PSUM accumulation
stays fp32. Measured-precision budget: harness gate is 2e-2 max-normalized.

On-device layouts (per core, S=2048, 4 heads, hd=64):
  xT      [1024, 2048] bf16  x[g].T               (d on partitions, chunked)
  QT/KT   2 x 4 x [128, 512] q^T/k^T per t-chunk, rows = 2 heads x 64, RoPE'd
  Vsb     4 x [128, 4*260]  v in [k, d] layout, key-blocks of
                            [v_h0|1|v_h1|1|v_h2|1|v_h3|1] (ones col -> den).
                            Produced DIRECTLY in [t, d] layout by swapping
                            matmul operands (lhsT = x chunk, rhs = w_v^T):
                            no PE transposes needed.
  scores  S^T [k, q] via matmul(lhsT=KT slice, rhs=QT slice); both heads of a
          pair write one shared [128,1024] psum tile (A cols 0:512, B cols
          512:1024) so a single ACT exp covers both; causal handled block-wise
          with ONE merged 3D-AP mask multiply for both streams
  ctx^T   accumulated [65, 512] psum per (head, q-chunk); row 64 = denominator
          (ones column appended to V); reciprocal read straight from PSUM,
          GpSimd partition-broadcast, one DVE multiply into bf16 ctxc
  out     ctxT [256, 2048] x w_outT -> partial [2048, 1024] bf16

Projection, attention, and output projection are emitted as interleaved unit
streams (attention(qc) with projection(qc+1), out-proj(qc) with attention
(qc+1)) so PE stays dense. Within an attention unit the AVs of the previous
block precede the scores (AV-B first: its exp wait subsumes AV-A's), leaving
the two K=64 score matmuls wait-free and adjacent — they execute concurrently
on PE row-groups 0/64. A burst of dependency-free warmup matmuls at kernel
start keeps the PE HAM busy during the initial DMA so real work runs at
K=8/8 (2.4 GHz) from the first tile.
"""

import numpy as np

DIM = 1024
NUM_HEADS = 16
HEAD_DIM = 64
ROPE_BASE = 10000.0
B = 2
S = 2048
N_CORES = 8
HPC = 4            # heads per core
GROUPS = 2         # batch groups
CPG = N_CORES // GROUPS  # cores per group
TC = 512           # t-chunk (tokens per projection chunk)
NTC = S // TC      # 4
KB = 128           # key block
NKB = S // KB      # 16
VW = 65            # v block width per head (64 + ones col)
VBW = HPC * VW     # 260
N_WARMUP = 44      # dependency-free PE warmup matmuls

_compiled = None


def _build_nc(debug_dump=False):
    import concourse.tile as tile
    from concourse import bacc, mybir
    from contextlib import ExitStack

    f32 = mybir.dt.float32
    bf16 = mybir.dt.bfloat16
    AF = mybir.ActivationFunctionType

    nc = bacc.Bacc("TRN2", target_bir_lowering=False, debug=False,
                   num_devices=N_CORES)

    xT = nc.dram_tensor("xT", [DIM, S], bf16, kind="ExternalInput").ap()
    wqkT = nc.dram_tensor("wqkT", [128, 8 * 512], bf16, kind="ExternalInput").ap()
    wvT = nc.dram_tensor("wvT", [128, 8 * 256], bf16, kind="ExternalInput").ap()
    woT = nc.dram_tensor("woT", [128, 2 * 1024], bf16, kind="ExternalInput").ap()
    cosT = nc.dram_tensor("cosT", [128, S], bf16, kind="ExternalInput").ap()
    sinT = nc.dram_tensor("sinT", [128, S], bf16, kind="ExternalInput").ap()
    perm = nc.dram_tensor("perm", [128, 128], bf16, kind="ExternalInput").ap()
    tri2 = nc.dram_tensor("tri2", [128, 256], bf16, kind="ExternalInput").ap()
    out_p = nc.dram_tensor("out_p", [S, DIM], bf16, kind="ExternalOutput").ap()
    if debug_dump:
        dbg_qt0 = nc.dram_tensor("dbg_qt0", [128, S], bf16, kind="ExternalOutput").ap()
        dbg_kt0 = nc.dram_tensor("dbg_kt0", [128, S], bf16, kind="ExternalOutput").ap()
        dbg_vsb = nc.dram_tensor("dbg_vsb", [128, NKB * VBW], bf16, kind="ExternalOutput").ap()
        dbg_ctx0 = nc.dram_tensor("dbg_ctx0", [128, S], bf16, kind="ExternalOutput").ap()
        dbg_cps = nc.dram_tensor("dbg_cps", [128, S], f32, kind="ExternalOutput").ap()
        dbg_bcr = nc.dram_tensor("dbg_bcr", [128, S], f32, kind="ExternalOutput").ap()

    with tile.TileContext(nc) as tc:
        with ExitStack() as ctx:
            big = ctx.enter_context(tc.tile_pool(name="big", bufs=1))

            # warmup tile first: memset has no DMA deps, so the warmup
            # matmuls below run while the first DMAs are still in flight
            warm_sb = big.tile([128, 128], bf16, tag="warm")
            nc.gpsimd.memset(warm_sb[:], 0.0)

            # DMA order matters: the queue drains serially, so interleave the
            # per-c8 wqk chunks with the first x chunks (the startup critical
            # path), then rope tables, then later-needed weights.
            # DMA order matters: two hardware DMA queues exist (sync + scalar).
            # Weights go on the scalar queue (idle at startup), x chunks on
            # sync, so the startup critical path drains two streams in
            # parallel.
            wqk_sb = [big.tile([128, 512], bf16, tag=f"wqk{c8}", name=f"wqk{c8}")
                      for c8 in range(8)]
            x0ts = []
            for c8 in range(8):
                nc.sync.dma_start(wqk_sb[c8][:], wqkT[:, c8 * 512:(c8 + 1) * 512])
                xt_t = big.tile([128, TC], bf16, tag=f"x0_{c8}", name=f"x{c8}_0")
                nc.sync.dma_start(xt_t[:], xT[c8 * 128:(c8 + 1) * 128, 0:TC])
                x0ts.append(xt_t)
            perm_sb = big.tile([128, 128], bf16, tag="perm")
            nc.sync.dma_start(perm_sb[:], perm[:])
            cos_sb = big.tile([128, S], bf16, tag="cos")
            nc.sync.dma_start(cos_sb[:], cosT[:])
            sin_sb = big.tile([128, S], bf16, tag="sin")
            nc.sync.dma_start(sin_sb[:], sinT[:])
            wv_sb = big.tile([128, 8 * 256], bf16, tag="wv")
            nc.sync.dma_start(wv_sb[:], wvT[:])
            tri2_sb = big.tile([128, 256], bf16, tag="tri2")
            nc.sync.dma_start(tri2_sb[:], tri2[:])
            wo_sb = big.tile([128, 2 * 1024], bf16, tag="wo")
            nc.sync.dma_start(wo_sb[:], woT[:])

            # per-chunk persistent tiles so phase 2 can start while phase 1
            # is still projecting later chunks
            qt = [[big.tile([128, TC], bf16, tag=f"qt{i}_{j}", name=f"qt{i}_{j}")
                   for j in range(NTC)] for i in range(2)]
            kt = [[big.tile([128, TC], bf16, tag=f"kt{i}_{j}", name=f"kt{i}_{j}")
                   for j in range(NTC)] for i in range(2)]
            vsb = [big.tile([128, 4 * VBW], bf16, tag=f"vsb{j}", name=f"vsb{j}")
                   for j in range(NTC)]
            ctxc = [[big.tile([128, TC], bf16, tag=f"ctx{i}_{j}", name=f"ctxc{i}_{j}")
                     for j in range(NTC)] for i in range(2)]

            if debug_dump:
                dbg_cps_sb = big.tile([128, S], f32, tag="dbgcps")
                dbg_bcr_sb = big.tile([128, S], f32, tag="dbgbcr")

            # ones columns interleaved in v blocks (denominator trick)
            ones128 = big.tile([128, 16], bf16, tag="ones128")
            nc.gpsimd.memset(ones128[:], 1.0)
            for j in range(NTC):
                nc.vector.tensor_copy(vsb[j][:, 64::VW], ones128[:])

            # ---- Fused phase 1 (projection) + phase 2 (attention) ----
            # attention for q-chunk qc only needs qt/kt/vsb chunks <= qc, so
            # attention(qc) is emitted interleaved with projection(qc+1).
            scale = float(HEAD_DIM) ** -0.5
            with (
                tc.tile_pool(name="p1x", bufs=2) as p1x,
                tc.tile_pool(name="p1sb", bufs=3) as p1sb,
                tc.tile_pool(name="p2sb", bufs=6) as p2sb,
                tc.tile_pool(name="pps", bufs=2, space="PSUM") as pps,
                tc.tile_pool(name="sps", bufs=2, space="PSUM") as sps,
                tc.tile_pool(name="ctxps", bufs=2, space="PSUM") as ctxps,
            ):
                rtps = pps

                # PE warmup: no data deps -> runs immediately, keeps the HAM
                # activity window busy while the first DMAs land
                warm_ps = pps.tile([128, 512], f32, tag="p", name="warm_ps")
                for _ in range(N_WARMUP):
                    nc.tensor.matmul(warm_ps[:, 0:128], warm_sb[:], warm_sb[:],
                                     start=True, stop=True)

                def p1_dma(tci):
                    if tci == 0:
                        return x0ts
                    xts = []
                    for c8 in range(8):
                        xt_t = p1x.tile([128, TC], bf16, tag=f"x{c8}", name=f"x{c8}_{tci}")
                        nc.sync.dma_start(
                            xt_t[:], xT[c8 * 128:(c8 + 1) * 128, tci * TC:(tci + 1) * TC])
                        xts.append(xt_t)
                    return xts

                def p1_qk_unit(xts, tci, blk):
                    cosw = cos_sb[:, tci * TC:(tci + 1) * TC]
                    sinw = sin_sb[:, tci * TC:(tci + 1) * TC]
                    qk_ps = pps.tile([128, TC], f32, tag="p", name=f"qk{tci}_{blk}")
                    for c8 in range(8):
                        nc.tensor.matmul(
                            qk_ps[:],
                            wqk_sb[c8][:, blk * 128:blk * 128 + 128],
                            xts[c8][:],
                            start=(c8 == 0), stop=(c8 == 7))
                    raw = p1sb.tile([128, TC], bf16, tag="raw", name=f"raw{tci}_{blk}")
                    # ACT drains psum (fast psum reads) so the projection's
                    # psum recycling doesn't ride on DVE's queue latency
                    nc.scalar.copy(raw[:], qk_ps[:])
                    rot_ps = rtps.tile([128, TC], f32, tag="p", name=f"rot{tci}_{blk}")
                    nc.tensor.matmul(rot_ps[:], perm_sb[:], raw[:], start=True, stop=True)
                    t1 = p1sb.tile([128, TC], bf16, tag="t1", name=f"t1_{tci}_{blk}")
                    nc.vector.tensor_mul(t1[:], raw[:], cosw)
                    t2 = p1sb.tile([128, TC], bf16, tag="t2", name=f"t2_{tci}_{blk}")
                    nc.vector.tensor_mul(t2[:], rot_ps[:], sinw)
                    dest = (qt if blk < 2 else kt)[blk % 2][tci]
                    nc.vector.tensor_add(dest[:], t1[:], t2[:])

                def p1_v_unit(xts, tci, tsub):
                    # V^T directly: out[t, vd] = (x chunk)^T @ w_v^T chunk,
                    # accumulating K=1024 over the 8 din chunks. lhsT is the
                    # x data (stationary), rhs the weight slice (moving).
                    v_ps = pps.tile([128, 256], f32, tag="p", name=f"vp{tci}_{tsub}")
                    for c8 in range(8):
                        nc.tensor.matmul(
                            v_ps[:],
                            xts[c8][:, tsub * 128:(tsub + 1) * 128],
                            wv_sb[:, c8 * 256:(c8 + 1) * 256],
                            start=(c8 == 0), stop=(c8 == 7))
                    # one strided cast into the per-head interleaved v layout
                    dst = vsb[tci][:, tsub * VBW:(tsub + 1) * VBW]
                    dst3 = dst.rearrange("p (h d) -> p h d", d=VW)[:, :, 0:64]
                    src3 = v_ps[:].rearrange("p (h d) -> p h d", d=64)
                    nc.vector.tensor_copy(dst3, src3)

                def p1_units(tci):
                    xts = p1_dma(tci)
                    units = []
                    for blk in range(4):
                        units.append(lambda b=blk: p1_qk_unit(xts, tci, b))
                    for tsub in range(4):
                        units.append(lambda t=tsub: p1_v_unit(xts, tci, t))
                    return units

                def att_scores(streams, kb):
                    """Both streams' score matmuls for k-block kb into ONE
                    shared psum tile (A cols 0:n, B cols 512:512+n). The two
                    K=64 matmuls at partition bases 0/64 are adjacent and
                    wait-free -> they run concurrently on PE row-groups,
                    restoring full SBUF-stream rate. One exp covers both."""
                    st0 = streams[0]
                    qc, d0 = st0["qc"], st0["qc"] * 4
                    n0 = max(0, 128 * (kb - d0))
                    n = 512 - n0
                    s_ps = sps.tile([128, 1024], f32, tag="s",
                                    name=f"s{st0['h']}_{qc}_{kb}")
                    e_sb = p2sb.tile([128, 1024], bf16, tag="e",
                                     name=f"e{st0['h']}_{qc}_{kb}", bufs=4)
                    for si_, st in enumerate(streams):
                        ti, po = st["ti"], st["po"]
                        off = 512 * si_
                        nc.tensor.matmul(
                            s_ps[:, off:off + n],
                            kt[ti][kb // 4][po:po + 64, (kb % 4) * 128:(kb % 4) * 128 + 128],
                            qt[ti][qc][po:po + 64, n0:512],
                            start=True, stop=True)
                    if n == 512:
                        nc.scalar.activation(e_sb[:, 0:1024], s_ps[:, 0:1024],
                                             AF.Exp, scale=scale)
                    else:
                        # one ACT over both streams' diag regions via 3D APs
                        ev = e_sb[:].rearrange("p (s q) -> p s q", s=2)[:, :, 0:n]
                        sv = s_ps[:].rearrange("p (s q) -> p s q", s=2)[:, :, 0:n]
                        nc.scalar.activation(ev, sv, AF.Exp, scale=scale)
                    if kb >= d0:
                        # one merged mask multiply for both streams: view the
                        # two 128-col diag regions as a [128, 2, 128] AP
                        ev = e_sb[:].rearrange("p (s q) -> p s q", s=2)[:, :, 0:128]
                        tv = tri2_sb[:].rearrange("p (s q) -> p s q", s=2)
                        nc.vector.tensor_mul(ev, ev, tv)
                    for si_, st in enumerate(streams):
                        st["pend"] = (kb, 512 * si_, n0, n, e_sb)

                def att_av(st):
                    h, qc = st["h"], st["qc"]
                    nkb = qc * 4 + 4
                    kb, o, n0, n, e_sb = st["pend"]
                    nc.tensor.matmul(
                        st["ctx"][0:65, n0:512],
                        vsb[kb // 4][:, (kb % 4) * VBW + VW * h:(kb % 4) * VBW + VW * h + VW],
                        e_sb[:, o:o + n],
                        start=(kb == 0), stop=(kb == nkb - 1))

                def att_finish(st):
                    h, qc, ti, po = st["h"], st["qc"], st["ti"], st["po"]
                    if debug_dump and h == 0:
                        nc.vector.tensor_copy(
                            dbg_cps_sb[0:65, qc * 512:(qc + 1) * 512], st["ctx"][0:65, :])
                    if qc == 3 and h >= 2:
                        # last pair: nothing waits on the psum bank; use the
                        # shortest chain to ctxc for the tail
                        den = p2sb.tile([1, 512], f32, tag="den", name=f"den{h}_{qc}", bufs=2)
                        nc.vector.tensor_copy(den[:], st["ctx"][64:65, :])
                        rden = p2sb.tile([1, 512], f32, tag="rden", name=f"rden{h}_{qc}", bufs=2)
                        nc.vector.reciprocal_approx_fast(rden[:], den[:])
                        bc_r = p2sb.tile([64, 512], f32, tag="bcr", name=f"bcr{h}_{qc}", bufs=2)
                        nc.gpsimd.partition_broadcast(bc_r[:], rden[:])
                        nc.vector.tensor_mul(
                            ctxc[ti][qc][po:po + 64, :], st["ctx"][0:64, :], bc_r[:])
                        return
                    # drain ctx psum to SBUF immediately on ACT: the bank
                    # frees after one copy, so the next pair's AVs don't wait
                    # for the whole normalize chain
                    ctx_sb = p2sb.tile([65, 512], bf16, tag="ctxs", name=f"ctxs{h}_{qc}", bufs=2)
                    nc.scalar.copy(ctx_sb[:], st["ctx"][0:65, :])
                    den = p2sb.tile([1, 512], f32, tag="den", name=f"den{h}_{qc}", bufs=2)
                    nc.vector.tensor_copy(den[:], ctx_sb[64:65, :])
                    rden = p2sb.tile([1, 512], f32, tag="rden", name=f"rden{h}_{qc}", bufs=2)
                    nc.vector.reciprocal_approx_fast(rden[:], den[:])
                    bc_r = p2sb.tile([64, 512], f32, tag="bcr", name=f"bcr{h}_{qc}", bufs=2)
                    nc.gpsimd.partition_broadcast(bc_r[:], rden[:])
                    nc.vector.tensor_mul(
                        ctxc[ti][qc][po:po + 64, :], ctx_sb[0:64, :], bc_r[:])

                def att_units(qc):
                    """Units for all 4 heads at q-chunk qc, two streams each.
                    AVs of pair pb-1 (stream B first: its exp wait subsumes
                    stream A's) precede scores of pb, so score matmuls carry
                    no fresh waits -> row-group pairing at bases 0/64."""
                    units = []
                    for hp in range(2):
                        streams = [{
                            "h": h, "qc": qc, "ti": h // 2, "po": 64 * (h % 2),
                        } for h in (2 * hp, 2 * hp + 1)]

                        def mk_start(strs=streams, q=qc):
                            def u():
                                # alloc B first: B's AVs run first and B's
                                # finish is emitted first, so the pool buf
                                # that frees earliest is the one the next
                                # pair needs earliest
                                for st in reversed(strs):
                                    st["ctx"] = ctxps.tile(
                                        [65, 512], f32, tag="ctx",
                                        name=f"cps{st['h']}_{q}")
                                    st["pend"] = None
                            return u
                        units.append(mk_start())

                        nkb = qc * 4 + 4
                        for kb in range(nkb):
                            def mk_unit(strs=streams, k=kb):
                                def u():
                                    if strs[0]["pend"] is not None:
                                        att_av(strs[1])  # B first: its wait
                                        att_av(strs[0])  # subsumes A's
                                    att_scores(strs, k)
                                return u
                            units.append(mk_unit())

                        def mk_tail(strs=streams):
                            def u():
                                att_av(strs[1])
                                att_av(strs[0])
                                att_finish(strs[1])
                                att_finish(strs[0])
                            return u
                        units.append(mk_tail())
                    return units

                osb_cur = {}

                def out_unit(tt, ec):
                    qc = tt // 4
                    o_ps = pps.tile([128, 512], f32, tag="p", name=f"o{tt}_{ec}")
                    for dc in range(2):
                        nc.tensor.matmul(
                            o_ps[:],
                            ctxc[dc][qc][:, (tt % 4) * 128:(tt % 4) * 128 + 128],
                            wo_sb[:, dc * 1024 + ec * 512:dc * 1024 + ec * 512 + 512],
                            start=(dc == 0), stop=(dc == 1))
                    if ec == 0:
                        osb_cur[tt] = p2sb.tile([128, 1024], bf16, tag="osb",
                                                name=f"ob{tt}", bufs=2)
                    o_sb = osb_cur[tt]
                    if qc == 3 and ec == 0:
                        # tail: exp work is over, ACT is idle -> split the
                        # serial cast chain across both engines
                        nc.scalar.copy(o_sb[:, 0:512], o_ps[:])
                    else:
                        nc.vector.tensor_copy(o_sb[:, ec * 512:(ec + 1) * 512], o_ps[:])
                    if qc == 3:
                        # per-half DMAs so the last transfer is small
                        nc.sync.dma_start(
                            out_p[tt * 128:(tt + 1) * 128, ec * 512:(ec + 1) * 512],
                            o_sb[:, ec * 512:(ec + 1) * 512])
                    elif ec == 1:
                        # one 256KB DMA per 128-row block (contiguous in dram)
                        nc.sync.dma_start(out_p[tt * 128:(tt + 1) * 128, :], o_sb[:])

                def out_units(qc):
                    return [lambda t=tt, e=ec: out_unit(t, e)
                            for tt in range(4 * qc, 4 * qc + 4) for ec in range(2)]

                def run_interleaved(a_units, b_units):
                    na, nb = len(a_units), len(b_units)
                    ia = ib = 0
                    while ia < na or ib < nb:
                        if ib >= nb or (ia < na and ia * nb <= ib * na):
                            a_units[ia](); ia += 1
                        else:
                            b_units[ib](); ib += 1

                run_interleaved(p1_units(0), [])
                run_interleaved(p1_units(1), att_units(0))
                run_interleaved(p1_units(2), att_units(1) + out_units(0))
                run_interleaved(p1_units(3), att_units(2) + out_units(1))
                run_interleaved(att_units(3), out_units(2))
                for u in out_units(3):
                    u()

            if debug_dump:
                for j in range(NTC):
                    nc.sync.dma_start(dbg_qt0[:, j * TC:(j + 1) * TC], qt[0][j][:])
                    nc.sync.dma_start(dbg_kt0[:, j * TC:(j + 1) * TC], kt[0][j][:])
                    nc.sync.dma_start(dbg_vsb[:, j * 4 * VBW:(j + 1) * 4 * VBW],
                                      vsb[j][:])
                    nc.sync.dma_start(dbg_ctx0[:, j * TC:(j + 1) * TC],
                                      ctxc[0][j][:])
                nc.sync.dma_start(dbg_cps[:], dbg_cps_sb[:])
                nc.sync.dma_start(dbg_bcr[:], dbg_bcr_sb[:])

    nc.compile()
    return nc


def _rope_tables():
    inv_freq = 1.0 / (ROPE_BASE ** (np.arange(0, HEAD_DIM, 2, dtype=np.float32) / HEAD_DIM))
    t = np.arange(S, dtype=np.float32)
    freqs = np.outer(t, inv_freq)                      # (S, 32)
    emb = np.concatenate([freqs, freqs], axis=-1)      # (S, 64)
    cos = np.cos(emb).astype(np.float32).T             # (64, S)
    sin = np.sin(emb).astype(np.float32).T
    return np.tile(cos, (2, 1)), np.tile(sin, (2, 1))  # (128, S)


def _perm_mat():
    p = np.zeros((128, 128), dtype=np.float32)
    for base in (0, 64):
        for d in range(32):
            p[base + d + 32, base + d] = -1.0          # rot[d] = -q[d+32]
        for d in range(32, 64):
            p[base + d - 32, base + d] = 1.0           # rot[d] = q[d-32]
    return p


def _bf16():
    from concourse import mybir
    return mybir.dt.np(mybir.dt.bfloat16)


def core_inputs(c, x, w_qkv, w_out, cos2, sin2, perm_np, tri2_np):
    bf = _bf16()
    g = c // CPG
    hs = [HPC * (c % CPG) + i for i in range(HPC)]
    xTg = np.ascontiguousarray(x[g].T).astype(bf)                        # (1024, 2048)

    qrows = np.concatenate([w_qkv[h * 64:(h + 1) * 64] for h in hs])     # (256, 1024)
    krows = np.concatenate([w_qkv[DIM + h * 64:DIM + (h + 1) * 64] for h in hs])
    vrows = np.concatenate([w_qkv[2 * DIM + h * 64:2 * DIM + (h + 1) * 64] for h in hs])
    wqk = np.concatenate([qrows, krows])                                 # (512, 1024)
    wqkT = np.ascontiguousarray(
        wqk.reshape(512, 8, 128).transpose(2, 1, 0).reshape(128, 8 * 512)).astype(bf)
    wvT = np.ascontiguousarray(
        vrows.reshape(256, 8, 128).transpose(2, 1, 0).reshape(128, 8 * 256)).astype(bf)

    didx = np.concatenate([np.arange(h * 64, (h + 1) * 64) for h in hs])  # (256,)
    woTh = w_out[:, didx].T                                               # (256, 1024)
    woT = np.ascontiguousarray(
        woTh.reshape(2, 128, DIM).transpose(1, 0, 2).reshape(128, 2 * DIM)).astype(bf)

    return {
        "xT": xTg, "wqkT": wqkT, "wvT": wvT, "woT": woT,
        "cosT": cos2, "sinT": sin2, "perm": perm_np, "tri2": tri2_np,
    }


def make_in_maps(x, w_qkv, w_out):
    bf = _bf16()
    x = np.asarray(x, dtype=np.float32)
    w_qkv = np.asarray(w_qkv, dtype=np.float32)
    w_out = np.asarray(w_out, dtype=np.float32)
    cos2, sin2 = _rope_tables()
    cos2 = cos2.astype(bf)
    sin2 = sin2.astype(bf)
    perm_np = _perm_mat().astype(bf)
    k_idx = np.arange(128)[:, None]
    q_idx = np.arange(128)[None, :]
    tri_np = (q_idx >= k_idx).astype(np.float32)
    tri2_np = np.concatenate([tri_np, tri_np], axis=1).astype(bf)        # (128, 256)
    return [core_inputs(c, x, w_qkv, w_out, cos2, sin2, perm_np, tri2_np)
            for c in range(N_CORES)]


def get_compiled():
    global _compiled
    if _compiled is None:
        _compiled = _build_nc()
    return _compiled


def gather(results):
    out = np.empty((B, S, DIM), dtype=np.float32)
    for g in range(GROUPS):
        acc = results[g * CPG]["out_p"].astype(np.float32)
        for c in range(g * CPG + 1, (g + 1) * CPG):
            acc += results[c]["out_p"].astype(np.float32)
        out[g] = acc
    return out


def kernel(x, w_qkv, w_out):
    from concourse.bass_utils import run_bass_kernel_spmd
    nc = get_compiled()
    in_maps = make_in_maps(x, w_qkv, w_out)
    res = run_bass_kernel_spmd(nc, in_maps, list(range(N_CORES)))
    return gather(res.results)
